# revision 18
# baseline (speedup 1.0000x reference)
"""Trainium2 Bass kernel for nn_Affinity (graph-matching affinity matrix).

Math per sample (validated against the reference):
  out[(a,c),(b,c')] = sum_{e2,e1} G2[a,e2] H2[b,e2] Me[e2,e1] G1[c,e1] H1[c',e1]
                      + diag(vec(Mp))

Device strategy (data-parallel, 1 sample per NeuronCore), fully static
instruction stream (no data-dependent control flow, no indirect DMA):
  1. Incidence G/H built on-device from A via a row-major exclusive prefix
     sum ("rank") of the threshold mask, one-hot expansion, and constant
     selection matmuls.
  2. Edge affinity Me, node affinity MpT via small matmuls.
  3. Z[e1,(a,b)] = Me^T-gather over g2 edges:  Z = Me @ P2 where
     P2[e2,(a,b)] = G2T[e2,a]*H2T[e2,b] (one-hot columns).
  4. Per output row-block a: out_a[c,(b,c')] = sum_e1 G1T[e1,c] * V_a[e1,(b,c')]
     with V_a = Z[:,32a:32a+32] (x) H1T — built on DVE in (c',b) order so both
     operands keep innermost stride 1 (fp16 2x mode); the matmul rhs AP reads
     it back in (b,c') order.  4 row-blocks share one PSUM tile via column
     tiling -> 128-partition PSUM->SBUF copies.  The diagonal is accumulated
     by one extra matmul against a shifted-identity constant.
  5. 32 per-row-block DMAs (4KB contiguous runs) write the 4MB output,
     overlapping the tail of the compute pipeline.
"""

import numpy as np

import concourse.bacc as bacc
import concourse.bass as bass
import concourse.mybir as mybir
import concourse.tile as tile
from concourse.bass_utils import run_bass_kernel_spmd

F32 = mybir.dt.float32
F16 = mybir.dt.float16
ALU = mybir.AluOpType
AX = mybir.AxisListType

B, N, D, E = 8, 32, 128, 96
NCORES = 8

# consts column layout (fp32 tensor)
C_ID = 0        # identity128          [:, 0:128]
C_SU = 128      # strictly-upper ones  [:, 128:256]
C_SELH = 256    # selhead (p//4==r)    [:, 256:288]
C_SELT = 288    # seltail chunks       [:, 288:544]
C_IOTA96 = 544  # per-row arange(96)   [:, 544:640]
C_W = 640
# fp16 consts: shifted identity, IDS[c, 512+c] = 1
CB_W = 1536


def make_consts():
    c = np.zeros((128, C_W), np.float32)
    c[:, C_ID:C_ID + 128] = np.eye(128)
    c[:, C_SU:C_SU + 128] = np.triu(np.ones((128, 128)), k=1)
    p = np.arange(128)
    c[:, C_SELH:C_SELH + 32] = (p[:, None] // 4 == np.arange(32)[None, :])
    for k in range(8):
        c[:, C_SELT + 32 * k:C_SELT + 32 * (k + 1)] = (
            8 * (p[:, None] % 4) + k == np.arange(32)[None, :])
    c[:, C_IOTA96:C_IOTA96 + 96] = np.arange(96)[None, :]
    cb = np.zeros((32, CB_W), np.float16)
    cb[np.arange(32), 512 + np.arange(32)] = 1.0
    return c, cb


def _incidence_both(nc, sb, ps, consts, ab):
    """Fused incidence build for both graphs. ab = (128, 16) = [A1f | A2f].

    Flat index f = p*8+k maps to A[f//32, f%32]; head r = p//4 and tail
    col = 8*(p%4)+k, so head/tail selection matrices are constants.
    Returns (32, 192) tiles gb = [G1 | G2], hb = [H1 | H2].
    """
    maskb = sb.tile([128, 16], F32, tag="maskb")
    nc.vector.tensor_scalar(out=maskb[:], in0=ab, scalar1=0.49, scalar2=None,
                            op0=ALU.is_ge)
    m3 = maskb[:].rearrange("p (g k) -> p g k", k=8)
    s2 = sb.tile([128, 2], F32, tag="s2")
    nc.vector.tensor_reduce(out=s2[:], in_=m3, axis=AX.X, op=ALU.add)
    pbase_ps = ps.tile([128, 2], F32, tag="psA", bufs=2)
    nc.tensor.matmul(out=pbase_ps[:], lhsT=consts[:, C_SU:C_SU + 128],
                     rhs=s2[:], start=True, stop=True)
    # inclusive prefix along k (8) by doubling, both graphs at once
    a = sb.tile([128, 16], F32, tag="pfa")
    b = sb.tile([128, 16], F32, tag="pfb")
    c = sb.tile([128, 16], F32, tag="pfc")
    a3, b3, c3 = (t[:].rearrange("p (g k) -> p g k", k=8) for t in (a, b, c))
    nc.vector.tensor_copy(out=a3[:, :, 0:1], in_=m3[:, :, 0:1])
    nc.vector.tensor_tensor(out=a3[:, :, 1:8], in0=m3[:, :, 1:8],
                            in1=m3[:, :, 0:7], op=ALU.add)
    nc.vector.tensor_copy(out=b3[:, :, 0:2], in_=a3[:, :, 0:2])
    nc.vector.tensor_tensor(out=b3[:, :, 2:8], in0=a3[:, :, 2:8],
                            in1=a3[:, :, 0:6], op=ALU.add)
    nc.vector.tensor_copy(out=c3[:, :, 0:4], in_=b3[:, :, 0:4])
    nc.vector.tensor_tensor(out=c3[:, :, 4:8], in0=b3[:, :, 4:8],
                            in1=b3[:, :, 0:4], op=ALU.add)
    # rank' = (incl + pbase) * mask - 1
    r0 = sb.tile([128, 16], F32, tag="r0")
    cb_, pb_ = bass.broadcast_tensor_aps(c3, pbase_ps[:, :].unsqueeze(2))
    nc.vector.tensor_tensor(out=r0[:].rearrange("p (g k) -> p g k", k=8),
                            in0=cb_, in1=pb_, op=ALU.add)
    r1 = sb.tile([128, 16], F32, tag="r1")
    nc.vector.tensor_tensor(out=r1[:], in0=r0[:], in1=maskb[:], op=ALU.mult)
    r2 = sb.tile([128, 16], F32, tag="r2")
    nc.vector.tensor_scalar(out=r2[:], in0=r1[:], scalar1=1.0, scalar2=None,
                            op0=ALU.subtract)
    # one-hot per (graph, k): oh[:, 192k + 96g : +96]
    oh = sb.tile([128, 16 * 96], F32, tag="oh")
    for k in range(8):
        for g in range(2):
            nc.vector.tensor_scalar(
                out=oh[:, 192 * k + 96 * g:192 * k + 96 * (g + 1)],
                in0=consts[:, C_IOTA96:C_IOTA96 + 96],
                scalar1=r2[:, 8 * g + k:8 * g + k + 1], scalar2=None,
                op0=ALU.is_equal)
    gps = ps.tile([32, 192], F32, tag="psacc", bufs=4)
    hps = ps.tile([32, 192], F32, tag="psacc", bufs=4)
    for k in range(8):
        nc.tensor.matmul(out=gps[:], lhsT=consts[:, C_SELH:C_SELH + 32],
                         rhs=oh[:, 192 * k:192 * (k + 1)],
                         start=(k == 0), stop=(k == 7))
    for k in range(8):
        nc.tensor.matmul(out=hps[:],
                         lhsT=consts[:, C_SELT + 32 * k:C_SELT + 32 * (k + 1)],
                         rhs=oh[:, 192 * k:192 * (k + 1)],
                         start=(k == 0), stop=(k == 7))
    gb = sb.tile([32, 192], F32, tag="gb")
    hb = sb.tile([32, 192], F32, tag="hb")
    nc.scalar.copy(out=gb[:], in_=gps[:])
    nc.scalar.copy(out=hb[:], in_=hps[:])
    return gb, hb


def build_program(debug: bool = False):
    nc = bacc.Bacc("TRN2", target_bir_lowering=False, debug=debug,
                   num_devices=NCORES)
    big0 = nc.dram_tensor("big0", [128, 336 + C_W], F32, kind="ExternalInput")
    big1 = nc.dram_tensor("big1", [32, 256], F32, kind="ExternalInput")
    cstb = nc.dram_tensor("cstb", [32, CB_W], F16, kind="ExternalInput")
    out = nc.dram_tensor("out", [32768, 32], F32, kind="ExternalOutput")

    with tile.TileContext(nc) as tc:
        with tc.tile_pool(name="sb", bufs=1) as sb, \
             tc.tile_pool(name="ps", bufs=1, space="PSUM") as ps:
            b0 = sb.tile([128, 336 + C_W], F32, tag="b0")
            nc.sync.dma_start(out=b0[:, 0:16], in_=big0[:, 0:16])
            nc.sync.dma_start(out=b0[:, 16:], in_=big0[:, 16:])
            b1 = sb.tile([32, 256], F32, tag="b1")
            nc.sync.dma_start(out=b1[:], in_=big1[:, :])
            ids16 = sb.tile([32, CB_W], F16, tag="ids16")
            nc.sync.dma_start(out=ids16[:], in_=cstb[:, :])
            a1sb, a2sb = b0[:, 0:8], b0[:, 8:16]
            u1sb, u2sb = b0[:, 16:48], b0[:, 48:80]
            l1sb, l2sb = b0[:, 80:208], b0[:, 208:336]
            consts = b0[:, 336:336 + C_W]
            ft1sb, ft2sb = b1[:, 0:128], b1[:, 128:256]
            ident = consts[:, C_ID:C_ID + 128]

            gb, hb = _incidence_both(nc, sb, ps, consts, b0[:, 0:16])
            g1sb, g2sb = gb[:, 0:96], gb[:, 96:192]
            h1sb, h2sb = hb[:, 0:96], hb[:, 96:192]

            # lam_i <- relu(lam_i + lam_i^T)  (symmetric)
            lp = []
            for i, lsb in enumerate((l1sb, l2sb)):
                ltps = ps.tile([128, 128], F32, tag="psA", bufs=2)
                nc.tensor.transpose(out=ltps[:], in_=lsb, identity=ident)
                lpi = sb.tile([128, 128], F32, tag=f"lp{i}")
                nc.vector.tensor_tensor(out=lpi[:], in0=lsb, in1=ltps[:],
                                        op=ALU.add)
                nc.vector.tensor_scalar(out=lpi[:], in0=lpi[:], scalar1=0.0,
                                        scalar2=None, op0=ALU.max)
                lp.append(lpi)

            # edge features X = [F1@G1; F1@H1], Y = [F2@G2; F2@H2] (128, 96)
            feats = {}
            for nm, ft_, gh in (("xg", ft1sb, g1sb), ("xh", ft1sb, h1sb),
                                ("yg", ft2sb, g2sb), ("yh", ft2sb, h2sb)):
                p_ = ps.tile([128, 96], F32, tag="psA", bufs=2)
                nc.tensor.matmul(out=p_[:], lhsT=ft_, rhs=gh,
                                 start=True, stop=True)
                s_ = sb.tile([128, 96], F32, tag=nm)
                nc.scalar.copy(out=s_[:], in_=p_[:])
                feats[nm] = s_

            # T1 = l1p@YG + l2p@YH ; T2 = l2p@YG + l1p@YH
            tts = []
            for i, (la, lb) in enumerate(((lp[0], lp[1]), (lp[1], lp[0]))):
                tp = ps.tile([128, 96], F32, tag="psA", bufs=2)
                nc.tensor.matmul(out=tp[:], lhsT=la[:], rhs=feats["yg"][:],
                                 start=True, stop=False)
                nc.tensor.matmul(out=tp[:], lhsT=lb[:], rhs=feats["yh"][:],
                                 start=False, stop=True)
                ts_ = sb.tile([128, 96], F32, tag=f"t{i}sb")
                nc.scalar.copy(out=ts_[:], in_=tp[:])
                tts.append(ts_)

            # Me[i, j] = sum_d XG[d,i] T1[d,j] + XH[d,i] T2[d,j]  (96, 96)
            meps = ps.tile([96, 96], F32, tag="psA", bufs=2)
            nc.tensor.matmul(out=meps[:], lhsT=feats["xg"][:], rhs=tts[0][:],
                             start=True, stop=False)
            nc.tensor.matmul(out=meps[:], lhsT=feats["xh"][:], rhs=tts[1][:],
                             start=False, stop=True)
            me16 = sb.tile([96, 96], F16, tag="me16")
            nc.scalar.copy(out=me16[:], in_=meps[:])

            # MpT[c, a] = Mp[a, c] = (U2^T U1)[c, a] -> fp16
            mptps = ps.tile([32, 32], F32, tag="psA", bufs=2)
            nc.tensor.matmul(out=mptps[:], lhsT=u2sb, rhs=u1sb,
                             start=True, stop=True)
            mpt16 = sb.tile([32, 32], F16, tag="mpt16")
            nc.scalar.copy(out=mpt16[:], in_=mptps[:])

            # transposes of incidences -> fp16 (96, 32)
            def transp16(src_, tag):
                tps = ps.tile([96, 32], F32, tag="psA", bufs=2)
                nc.tensor.transpose(out=tps[:], in_=src_,
                                    identity=consts[0:32, C_ID:C_ID + 32])
                t16 = sb.tile([96, 32], F16, tag=tag)
                nc.scalar.copy(out=t16[:], in_=tps[:])
                return t16
            g1t16 = transp16(g1sb, "g1t16")
            h1t16 = transp16(h1sb, "h1t16")
            g2t16 = transp16(g2sb, "g2t16")
            h2t16 = transp16(h2sb, "h2t16")

            # P2[e2, (a, b)] = G2T[e2, a] * H2T[e2, b]   (96, 1024) fp16
            p2 = sb.tile([96, 1024], F16, tag="p2")
            g2b, h2b = bass.broadcast_tensor_aps(g2t16[:, :].unsqueeze(2),
                                                 h2t16[:, :].unsqueeze(1))
            nc.vector.tensor_tensor(
                out=p2[:].rearrange("p (a b) -> p a b", b=32),
                in0=g2b, in1=h2b, op=ALU.mult)

            # Z[e1, (a, b)] = sum_e2 Me[e2, e1] P2[e2, (a,b)]  -> fp16
            z16 = sb.tile([96, 1024], F16, tag="z16")
            for h in range(2):
                zps = ps.tile([96, 512], F32, tag="psA", bufs=2)
                nc.tensor.matmul(out=zps[:], lhsT=me16[:],
                                 rhs=p2[:, 512 * h:512 * (h + 1)],
                                 start=True, stop=True)
                nc.scalar.copy(out=z16[:, 512 * h:512 * (h + 1)], in_=zps[:])

            # H1Texp[e1, (c', b)] = H1T[e1, c']   (96, 1024) fp16
            h1exp = sb.tile([96, 1024], F16, tag="h1exp")
            nc.gpsimd.tensor_copy(
                out=h1exp[:].rearrange("p (c b) -> p c b", b=32),
                in_=h1t16[:, :].unsqueeze(2).broadcast_to([96, 32, 32]))

            # D16[c, (a, c')] = eye[c, c'] * MpT[c, a]  (32, 1024) fp16
            d16 = sb.tile([32, 1024], F16, tag="d16")
            eyb, mpb = bass.broadcast_tensor_aps(
                ids16[:, 512:544].unsqueeze(1), mpt16[:, :].unsqueeze(2))
            nc.gpsimd.tensor_tensor(
                out=d16[:].rearrange("p (a c) -> p a c", c=32),
                in0=eyb, in1=mpb, op=ALU.mult)

            # obuf[(q, c), g*1024 + (b, c')] = out row-block alpha = 4g+q
            obuf = sb.tile([128, 8192], F32, tag="obuf")
            for g in range(8):
                # V4 = [V_a for a in 4g..4g+4], each (96, 1024) in (c', b) order
                v4 = sb.tile([96, 4096], F16, tag="v4", bufs=3)
                zap = z16[:, :]
                zin = bass.AP(zap.tensor, zap.offset + 128 * g,
                              [zap.ap[0], [32, 4], [0, 32], [1, 32]])
                hap = h1exp[:, :]
                hin = bass.AP(hap.tensor, hap.offset,
                              [hap.ap[0], [0, 4], [32, 32], [1, 32]])
                veng = nc.vector
                veng.tensor_tensor(
                    out=v4[:].rearrange("p (a c b) -> p a c b", a=4, b=32),
                    in0=zin, in1=hin, op=ALU.mult)
                for h in range(2):
                    pso = ps.tile([128, 512], F32, tag="pso", bufs=2)
                    for q in range(4):
                        alpha = 4 * g + q
                        has_diag = (alpha // 16) == h
                        # rhs: V_alpha read in (b, c') order, b in [16h,16h+16)
                        va = v4[:, 1024 * q:1024 * (q + 1)].rearrange(
                            "p (c b) -> p c b", b=32).transpose([0, 2, 1])
                        nc.tensor.matmul(out=pso[32 * q:32 * (q + 1), :],
                                         lhsT=g1t16[:],
                                         rhs=va[:, 16 * h:16 * (h + 1), :],
                                         start=True, stop=not has_diag,
                                         tile_position=(0, 32 * q))
                        if has_diag:
                            p_ = alpha % 16
                            nc.tensor.matmul(
                                out=pso[32 * q:32 * (q + 1), :],
                                lhsT=d16[:, 32 * alpha:32 * (alpha + 1)],
                                rhs=ids16[:, 512 - 32 * p_:1024 - 32 * p_],
                                start=False, stop=True,
                                tile_position=(0, 32 * q))
                    dst = obuf[:, 1024 * g + 512 * h:1024 * g + 512 * (h + 1)]
                    nc.scalar.copy(out=dst, in_=pso[:])

            # final DMAs: row-block alpha = 4g+q lives at
            # obuf[32q:32q+32, 1024g:1024(g+1)] -> out rows alpha*32..+32
            for g in range(8):
                for q in range(4):
                    alpha = 4 * g + q
                    dst = bass.AP(out, alpha * 32768,
                                  [[1024, 32], [1, 1024]])
                    nc.sync.dma_start(
                        out=dst,
                        in_=obuf[32 * q:32 * (q + 1),
                                 1024 * g:1024 * (g + 1)])
    nc.compile()
    return nc


def make_in_maps(inputs: dict) -> list:
    inputs = {k: np.asarray(v, dtype=np.float32) for k, v in inputs.items()}
    consts, constsb = make_consts()
    in_maps = []
    for b in range(B):
        big0 = np.concatenate([
            inputs["A_src"][b].reshape(128, 8).astype(np.float32),
            inputs["A_tgt"][b].reshape(128, 8).astype(np.float32),
            inputs["U_src"][b].astype(np.float32),
            inputs["U_tgt"][b].astype(np.float32),
            inputs["lambda1"].astype(np.float32),
            inputs["lambda2"].astype(np.float32),
            consts,
        ], axis=1)
        big1 = np.concatenate([
            inputs["F_src"][b].T.astype(np.float32),
            inputs["F_tgt"][b].T.astype(np.float32),
        ], axis=1)
        in_maps.append({
            "big0": np.ascontiguousarray(big0),
            "big1": np.ascontiguousarray(big1),
            "cstb": constsb,
        })
    return in_maps


_NC_CACHE = {}


def kernel(trace: bool = False, **inputs) -> np.ndarray:
    if "nc" not in _NC_CACHE:
        _NC_CACHE["nc"] = build_program()
    nc = _NC_CACHE["nc"]
    in_maps = make_in_maps(inputs)
    res = run_bass_kernel_spmd(nc, in_maps, core_ids=list(range(NCORES)),
                               trace=trace)
    _NC_CACHE["last_results"] = res
    outs = [res.results[b]["out"].reshape(1024, 1024) for b in range(B)]
    return np.stack(outs).astype(np.float32)


# revision 19
# speedup vs baseline: 1.0242x; 1.0242x over previous
"""Trainium2 Bass kernel for nn_Affinity (graph-matching affinity matrix).

Math per sample (validated against the reference):
  out[(a,c),(b,c')] = sum_{e2,e1} G2[a,e2] H2[b,e2] Me[e2,e1] G1[c,e1] H1[c',e1]
                      + diag(vec(Mp))

Device strategy (data-parallel, 1 sample per NeuronCore), fully static
instruction stream (no data-dependent control flow, no indirect DMA):
  1. Incidence G/H built on-device from A via a row-major exclusive prefix
     sum ("rank") of the threshold mask, one-hot expansion, and constant
     selection matmuls.
  2. Edge affinity Me, node affinity MpT via small matmuls.
  3. Z[e1,(a,b)] = Me^T-gather over g2 edges:  Z = Me @ P2 where
     P2[e2,(a,b)] = G2T[e2,a]*H2T[e2,b] (one-hot columns).
  4. Per output row-block a: out_a[c,(b,c')] = sum_e1 G1T[e1,c] * V_a[e1,(b,c')]
     with V_a = Z[:,32a:32a+32] (x) H1T — built on DVE in (c',b) order so both
     operands keep innermost stride 1 (fp16 2x mode); the matmul rhs AP reads
     it back in (b,c') order.  4 row-blocks share one PSUM tile via column
     tiling -> 128-partition PSUM->SBUF copies.  The diagonal is accumulated
     by one extra matmul against a shifted-identity constant.
  5. 32 per-row-block DMAs (4KB contiguous runs) write the 4MB output,
     overlapping the tail of the compute pipeline.
"""

import numpy as np

import concourse.bacc as bacc
import concourse.bass as bass
import concourse.mybir as mybir
import concourse.tile as tile
from concourse.bass_utils import run_bass_kernel_spmd

F32 = mybir.dt.float32
F16 = mybir.dt.float16
ALU = mybir.AluOpType
AX = mybir.AxisListType

B, N, D, E = 8, 32, 128, 96
NCORES = 8

# consts column layout (fp32 tensor)
C_ID = 0        # identity128          [:, 0:128]
C_SU = 128      # strictly-upper ones  [:, 128:256]
C_SELH = 256    # selhead (p//4==r)    [:, 256:288]
C_SELT = 288    # seltail chunks       [:, 288:544]
C_IOTA96 = 544  # per-row arange(96)   [:, 544:640]
C_W = 640
# fp16 consts: shifted identity, IDS[c, 512+c] = 1
CB_W = 1536


def make_consts():
    c = np.zeros((128, C_W), np.float32)
    c[:, C_ID:C_ID + 128] = np.eye(128)
    c[:, C_SU:C_SU + 128] = np.triu(np.ones((128, 128)), k=1)
    p = np.arange(128)
    c[:, C_SELH:C_SELH + 32] = (p[:, None] // 4 == np.arange(32)[None, :])
    for k in range(8):
        c[:, C_SELT + 32 * k:C_SELT + 32 * (k + 1)] = (
            8 * (p[:, None] % 4) + k == np.arange(32)[None, :])
    c[:, C_IOTA96:C_IOTA96 + 96] = np.arange(96)[None, :]
    cb = np.zeros((32, CB_W), np.float16)
    cb[np.arange(32), 512 + np.arange(32)] = 1.0
    return c, cb


def _incidence_both(nc, sb, ps, consts, ab):
    """Fused incidence build for both graphs. ab = (128, 16) = [A1f | A2f].

    Flat index f = p*8+k maps to A[f//32, f%32]; head r = p//4 and tail
    col = 8*(p%4)+k, so head/tail selection matrices are constants.
    Returns (32, 192) tiles gb = [G1 | G2], hb = [H1 | H2].
    """
    maskb = sb.tile([128, 16], F32, tag="maskb")
    nc.vector.tensor_scalar(out=maskb[:], in0=ab, scalar1=0.49, scalar2=None,
                            op0=ALU.is_ge)
    m3 = maskb[:].rearrange("p (g k) -> p g k", k=8)
    s2 = sb.tile([128, 2], F32, tag="s2")
    nc.vector.tensor_reduce(out=s2[:], in_=m3, axis=AX.X, op=ALU.add)
    pbase_ps = ps.tile([128, 2], F32, tag="psA", bufs=2)
    nc.tensor.matmul(out=pbase_ps[:], lhsT=consts[:, C_SU:C_SU + 128],
                     rhs=s2[:], start=True, stop=True)
    # inclusive prefix along k (8) by doubling, both graphs at once
    a = sb.tile([128, 16], F32, tag="pfa")
    b = sb.tile([128, 16], F32, tag="pfb")
    c = sb.tile([128, 16], F32, tag="pfc")
    a3, b3, c3 = (t[:].rearrange("p (g k) -> p g k", k=8) for t in (a, b, c))
    nc.vector.tensor_copy(out=a3[:, :, 0:1], in_=m3[:, :, 0:1])
    nc.vector.tensor_tensor(out=a3[:, :, 1:8], in0=m3[:, :, 1:8],
                            in1=m3[:, :, 0:7], op=ALU.add)
    nc.vector.tensor_copy(out=b3[:, :, 0:2], in_=a3[:, :, 0:2])
    nc.vector.tensor_tensor(out=b3[:, :, 2:8], in0=a3[:, :, 2:8],
                            in1=a3[:, :, 0:6], op=ALU.add)
    nc.vector.tensor_copy(out=c3[:, :, 0:4], in_=b3[:, :, 0:4])
    nc.vector.tensor_tensor(out=c3[:, :, 4:8], in0=b3[:, :, 4:8],
                            in1=b3[:, :, 0:4], op=ALU.add)
    # rank' = (incl + pbase) * mask - 1
    r0 = sb.tile([128, 16], F32, tag="r0")
    cb_, pb_ = bass.broadcast_tensor_aps(c3, pbase_ps[:, :].unsqueeze(2))
    nc.vector.tensor_tensor(out=r0[:].rearrange("p (g k) -> p g k", k=8),
                            in0=cb_, in1=pb_, op=ALU.add)
    r1 = sb.tile([128, 16], F32, tag="r1")
    nc.vector.tensor_tensor(out=r1[:], in0=r0[:], in1=maskb[:], op=ALU.mult)
    r2 = sb.tile([128, 16], F32, tag="r2")
    nc.vector.tensor_scalar(out=r2[:], in0=r1[:], scalar1=1.0, scalar2=None,
                            op0=ALU.subtract)
    # one-hot per (graph, k): oh[:, 192k + 96g : +96]
    oh = sb.tile([128, 16 * 96], F32, tag="oh")
    for k in range(8):
        for g in range(2):
            nc.vector.tensor_scalar(
                out=oh[:, 192 * k + 96 * g:192 * k + 96 * (g + 1)],
                in0=consts[:, C_IOTA96:C_IOTA96 + 96],
                scalar1=r2[:, 8 * g + k:8 * g + k + 1], scalar2=None,
                op0=ALU.is_equal)
    gps = ps.tile([32, 192], F32, tag="psacc", bufs=4)
    hps = ps.tile([32, 192], F32, tag="psacc", bufs=4)
    for k in range(8):
        nc.tensor.matmul(out=gps[:], lhsT=consts[:, C_SELH:C_SELH + 32],
                         rhs=oh[:, 192 * k:192 * (k + 1)],
                         start=(k == 0), stop=(k == 7))
    for k in range(8):
        nc.tensor.matmul(out=hps[:],
                         lhsT=consts[:, C_SELT + 32 * k:C_SELT + 32 * (k + 1)],
                         rhs=oh[:, 192 * k:192 * (k + 1)],
                         start=(k == 0), stop=(k == 7))
    gb = sb.tile([32, 192], F32, tag="gb")
    hb = sb.tile([32, 192], F32, tag="hb")
    nc.scalar.copy(out=gb[:], in_=gps[:])
    nc.scalar.copy(out=hb[:], in_=hps[:])
    return gb, hb


def build_program(debug: bool = False):
    nc = bacc.Bacc("TRN2", target_bir_lowering=False, debug=debug,
                   num_devices=NCORES)
    big0 = nc.dram_tensor("big0", [128, 336 + C_W], F32, kind="ExternalInput")
    big1 = nc.dram_tensor("big1", [32, 256], F32, kind="ExternalInput")
    cstb = nc.dram_tensor("cstb", [32, CB_W], F16, kind="ExternalInput")
    out = nc.dram_tensor("out", [32768, 32], F32, kind="ExternalOutput")

    with tile.TileContext(nc) as tc:
        with tc.tile_pool(name="sb", bufs=1) as sb, \
             tc.tile_pool(name="ps", bufs=1, space="PSUM") as ps:
            b0 = sb.tile([128, 336 + C_W], F32, tag="b0")
            nc.sync.dma_start(out=b0[:, 0:16], in_=big0[:, 0:16])
            nc.sync.dma_start(out=b0[:, 16:], in_=big0[:, 16:])
            b1 = sb.tile([32, 256], F32, tag="b1")
            nc.sync.dma_start(out=b1[:], in_=big1[:, :])
            ids16 = sb.tile([32, CB_W], F16, tag="ids16")
            nc.sync.dma_start(out=ids16[:], in_=cstb[:, :])
            a1sb, a2sb = b0[:, 0:8], b0[:, 8:16]
            u1sb, u2sb = b0[:, 16:48], b0[:, 48:80]
            l1sb, l2sb = b0[:, 80:208], b0[:, 208:336]
            consts = b0[:, 336:336 + C_W]
            ft1sb, ft2sb = b1[:, 0:128], b1[:, 128:256]
            ident = consts[:, C_ID:C_ID + 128]

            gb, hb = _incidence_both(nc, sb, ps, consts, b0[:, 0:16])
            g1sb, g2sb = gb[:, 0:96], gb[:, 96:192]
            h1sb, h2sb = hb[:, 0:96], hb[:, 96:192]

            # lam_i <- relu(lam_i + lam_i^T)  (symmetric)
            lp = []
            for i, lsb in enumerate((l1sb, l2sb)):
                ltps = ps.tile([128, 128], F32, tag="psA", bufs=2)
                nc.tensor.transpose(out=ltps[:], in_=lsb, identity=ident)
                lpi = sb.tile([128, 128], F32, tag=f"lp{i}")
                nc.vector.tensor_tensor(out=lpi[:], in0=lsb, in1=ltps[:],
                                        op=ALU.add)
                nc.vector.tensor_scalar(out=lpi[:], in0=lpi[:], scalar1=0.0,
                                        scalar2=None, op0=ALU.max)
                lp.append(lpi)

            # edge features X = [F1@G1; F1@H1], Y = [F2@G2; F2@H2] (128, 96)
            feats = {}
            for nm, ft_, gh in (("xg", ft1sb, g1sb), ("xh", ft1sb, h1sb),
                                ("yg", ft2sb, g2sb), ("yh", ft2sb, h2sb)):
                p_ = ps.tile([128, 96], F32, tag="psA", bufs=2)
                nc.tensor.matmul(out=p_[:], lhsT=ft_, rhs=gh,
                                 start=True, stop=True)
                s_ = sb.tile([128, 96], F32, tag=nm)
                nc.scalar.copy(out=s_[:], in_=p_[:])
                feats[nm] = s_

            # T1 = l1p@YG + l2p@YH ; T2 = l2p@YG + l1p@YH
            tts = []
            for i, (la, lb) in enumerate(((lp[0], lp[1]), (lp[1], lp[0]))):
                tp = ps.tile([128, 96], F32, tag="psA", bufs=2)
                nc.tensor.matmul(out=tp[:], lhsT=la[:], rhs=feats["yg"][:],
                                 start=True, stop=False)
                nc.tensor.matmul(out=tp[:], lhsT=lb[:], rhs=feats["yh"][:],
                                 start=False, stop=True)
                ts_ = sb.tile([128, 96], F32, tag=f"t{i}sb")
                nc.scalar.copy(out=ts_[:], in_=tp[:])
                tts.append(ts_)

            # Me[i, j] = sum_d XG[d,i] T1[d,j] + XH[d,i] T2[d,j]  (96, 96)
            meps = ps.tile([96, 96], F32, tag="psA", bufs=2)
            nc.tensor.matmul(out=meps[:], lhsT=feats["xg"][:], rhs=tts[0][:],
                             start=True, stop=False)
            nc.tensor.matmul(out=meps[:], lhsT=feats["xh"][:], rhs=tts[1][:],
                             start=False, stop=True)
            me16 = sb.tile([96, 96], F16, tag="me16")
            nc.scalar.copy(out=me16[:], in_=meps[:])

            # MpT[c, a] = Mp[a, c] = (U2^T U1)[c, a] -> fp16
            mptps = ps.tile([32, 32], F32, tag="psA", bufs=2)
            nc.tensor.matmul(out=mptps[:], lhsT=u2sb, rhs=u1sb,
                             start=True, stop=True)
            mpt16 = sb.tile([32, 32], F16, tag="mpt16")
            nc.scalar.copy(out=mpt16[:], in_=mptps[:])

            # transposes of incidences -> fp16 (96, 32)
            def transp16(src_, tag):
                tps = ps.tile([96, 32], F32, tag="psA", bufs=2)
                nc.tensor.transpose(out=tps[:], in_=src_,
                                    identity=consts[0:32, C_ID:C_ID + 32])
                t16 = sb.tile([96, 32], F16, tag=tag)
                nc.scalar.copy(out=t16[:], in_=tps[:])
                return t16
            g1t16 = transp16(g1sb, "g1t16")
            h1t16 = transp16(h1sb, "h1t16")
            g2t16 = transp16(g2sb, "g2t16")
            h2t16 = transp16(h2sb, "h2t16")

            # P2[e2, (a, b)] = G2T[e2, a] * H2T[e2, b]   (96, 1024) fp16
            p2 = sb.tile([96, 1024], F16, tag="p2")
            g2b, h2b = bass.broadcast_tensor_aps(g2t16[:, :].unsqueeze(2),
                                                 h2t16[:, :].unsqueeze(1))
            nc.vector.tensor_tensor(
                out=p2[:].rearrange("p (a b) -> p a b", b=32),
                in0=g2b, in1=h2b, op=ALU.mult)

            # Z[e1, (a, b)] = sum_e2 Me[e2, e1] P2[e2, (a,b)]  -> fp16
            z16 = sb.tile([96, 1024], F16, tag="z16")
            for h in range(2):
                zps = ps.tile([96, 512], F32, tag="psA", bufs=2)
                nc.tensor.matmul(out=zps[:], lhsT=me16[:],
                                 rhs=p2[:, 512 * h:512 * (h + 1)],
                                 start=True, stop=True)
                nc.scalar.copy(out=z16[:, 512 * h:512 * (h + 1)], in_=zps[:])

            # H1Texp[e1, (c', b)] = H1T[e1, c']   (96, 1024) fp16
            h1exp = sb.tile([96, 1024], F16, tag="h1exp")
            nc.gpsimd.tensor_copy(
                out=h1exp[:].rearrange("p (c b) -> p c b", b=32),
                in_=h1t16[:, :].unsqueeze(2).broadcast_to([96, 32, 32]))

            # D16[c, (a, c')] = eye[c, c'] * MpT[c, a]  (32, 1024) fp16
            d16 = sb.tile([32, 1024], F16, tag="d16")
            eyb, mpb = bass.broadcast_tensor_aps(
                ids16[:, 512:544].unsqueeze(1), mpt16[:, :].unsqueeze(2))
            nc.gpsimd.tensor_tensor(
                out=d16[:].rearrange("p (a c) -> p a c", c=32),
                in0=eyb, in1=mpb, op=ALU.mult)

            # obuf[(q, c), g*1024 + (b, c')] = out row-block alpha = 4g+q
            obuf = sb.tile([128, 8192], F32, tag="obuf")
            for g in range(8):
                # V4 = [V_a for a in 4g..4g+4], each (96, 1024) in (c', b) order
                v4 = sb.tile([96, 4096], F16, tag="v4", bufs=3)
                zap = z16[:, :]
                zin = bass.AP(zap.tensor, zap.offset + 128 * g,
                              [zap.ap[0], [32, 4], [0, 32], [1, 32]])
                hap = h1exp[:, :]
                hin = bass.AP(hap.tensor, hap.offset,
                              [hap.ap[0], [0, 4], [32, 32], [1, 32]])
                veng = nc.vector
                veng.tensor_tensor(
                    out=v4[:].rearrange("p (a c b) -> p a c b", a=4, b=32),
                    in0=zin, in1=hin, op=ALU.mult)
                for h in range(2):
                    pso = ps.tile([128, 512], F32, tag="pso", bufs=2)
                    for q in range(4):
                        alpha = 4 * g + q
                        has_diag = (alpha // 16) == h
                        # rhs: V_alpha read in (b, c') order, b in [16h,16h+16)
                        va = v4[:, 1024 * q:1024 * (q + 1)].rearrange(
                            "p (c b) -> p c b", b=32).transpose([0, 2, 1])
                        nc.tensor.matmul(out=pso[32 * q:32 * (q + 1), :],
                                         lhsT=g1t16[:],
                                         rhs=va[:, 16 * h:16 * (h + 1), :],
                                         start=True, stop=not has_diag,
                                         tile_position=(0, 32 * q))
                        if has_diag:
                            p_ = alpha % 16
                            nc.tensor.matmul(
                                out=pso[32 * q:32 * (q + 1), :],
                                lhsT=d16[:, 32 * alpha:32 * (alpha + 1)],
                                rhs=ids16[:, 512 - 32 * p_:1024 - 32 * p_],
                                start=False, stop=True,
                                tile_position=(0, 32 * q))
                    dst = obuf[:, 1024 * g + 512 * h:1024 * g + 512 * (h + 1)]
                    nc.scalar.copy(out=dst, in_=pso[:])

            # final DMAs: group g covers contiguous out rows [128g, 128g+128)
            # (alpha = 4g+q -> rows (4g+q)*32+c = 128g + 32q + c, and the
            # obuf partition order (q, c) matches the dst row order), so each
            # group writes one fully contiguous 512KB block.
            for g in range(8):
                dst = bass.AP(out, g * 131072, [[1024, 128], [1, 1024]])
                nc.sync.dma_start(out=dst,
                                  in_=obuf[:, 1024 * g:1024 * (g + 1)])
    nc.compile()
    return nc


def make_in_maps(inputs: dict) -> list:
    inputs = {k: np.asarray(v, dtype=np.float32) for k, v in inputs.items()}
    consts, constsb = make_consts()
    in_maps = []
    for b in range(B):
        big0 = np.concatenate([
            inputs["A_src"][b].reshape(128, 8).astype(np.float32),
            inputs["A_tgt"][b].reshape(128, 8).astype(np.float32),
            inputs["U_src"][b].astype(np.float32),
            inputs["U_tgt"][b].astype(np.float32),
            inputs["lambda1"].astype(np.float32),
            inputs["lambda2"].astype(np.float32),
            consts,
        ], axis=1)
        big1 = np.concatenate([
            inputs["F_src"][b].T.astype(np.float32),
            inputs["F_tgt"][b].T.astype(np.float32),
        ], axis=1)
        in_maps.append({
            "big0": np.ascontiguousarray(big0),
            "big1": np.ascontiguousarray(big1),
            "cstb": constsb,
        })
    return in_maps


_NC_CACHE = {}


def kernel(trace: bool = False, **inputs) -> np.ndarray:
    if "nc" not in _NC_CACHE:
        _NC_CACHE["nc"] = build_program()
    nc = _NC_CACHE["nc"]
    in_maps = make_in_maps(inputs)
    res = run_bass_kernel_spmd(nc, in_maps, core_ids=list(range(NCORES)),
                               trace=trace)
    _NC_CACHE["last_results"] = res
    outs = [res.results[b]["out"].reshape(1024, 1024) for b in range(B)]
    return np.stack(outs).astype(np.float32)


# revision 20
# speedup vs baseline: 1.0262x; 1.0019x over previous
"""Trainium2 Bass kernel for nn_Affinity (graph-matching affinity matrix).

Math per sample (validated against the reference):
  out[(a,c),(b,c')] = sum_{e2,e1} G2[a,e2] H2[b,e2] Me[e2,e1] G1[c,e1] H1[c',e1]
                      + diag(vec(Mp))

Device strategy (data-parallel, 1 sample per NeuronCore), fully static
instruction stream (no data-dependent control flow, no indirect DMA):
  1. Incidence G/H built on-device from A via a row-major exclusive prefix
     sum ("rank") of the threshold mask, one-hot expansion, and constant
     selection matmuls.
  2. Edge affinity Me, node affinity MpT via small matmuls.
  3. Z[e1,(a,b)] = Me^T-gather over g2 edges:  Z = Me @ P2 where
     P2[e2,(a,b)] = G2T[e2,a]*H2T[e2,b] (one-hot columns).
  4. Per output row-block a: out_a[c,(b,c')] = sum_e1 G1T[e1,c] * V_a[e1,(b,c')]
     with V_a = Z[:,32a:32a+32] (x) H1T — built on DVE in (c',b) order so both
     operands keep innermost stride 1 (fp16 2x mode); the matmul rhs AP reads
     it back in (b,c') order.  4 row-blocks share one PSUM tile via column
     tiling -> 128-partition PSUM->SBUF copies.  The diagonal is accumulated
     by one extra matmul against a shifted-identity constant.
  5. 32 per-row-block DMAs (4KB contiguous runs) write the 4MB output,
     overlapping the tail of the compute pipeline.
"""

import numpy as np

import concourse.bacc as bacc
import concourse.bass as bass
import concourse.mybir as mybir
import concourse.tile as tile
from concourse.bass_utils import run_bass_kernel_spmd

F32 = mybir.dt.float32
F16 = mybir.dt.float16
ALU = mybir.AluOpType
AX = mybir.AxisListType

B, N, D, E = 8, 32, 128, 96
NCORES = 8

# consts column layout (fp32 tensor)
C_ID = 0        # identity128          [:, 0:128]
C_SU = 128      # strictly-upper ones  [:, 128:256]
C_SELH = 256    # selhead (p//4==r)    [:, 256:288]
C_SELT = 288    # seltail chunks       [:, 288:544]
C_IOTA96 = 544  # per-row arange(96)   [:, 544:640]
C_W = 640
# fp16 consts: shifted identity, IDS[c, 512+c] = 1
CB_W = 1536


def make_consts():
    c = np.zeros((128, C_W), np.float32)
    c[:, C_ID:C_ID + 128] = np.eye(128)
    c[:, C_SU:C_SU + 128] = np.triu(np.ones((128, 128)), k=1)
    p = np.arange(128)
    c[:, C_SELH:C_SELH + 32] = (p[:, None] // 4 == np.arange(32)[None, :])
    for k in range(8):
        c[:, C_SELT + 32 * k:C_SELT + 32 * (k + 1)] = (
            8 * (p[:, None] % 4) + k == np.arange(32)[None, :])
    c[:, C_IOTA96:C_IOTA96 + 96] = np.arange(96)[None, :]
    cb = np.zeros((32, CB_W), np.float16)
    cb[np.arange(32), 512 + np.arange(32)] = 1.0
    return c, cb


def _incidence_both(nc, sb, ps, consts, ab):
    """Fused incidence build for both graphs. ab = (128, 16) = [A1f | A2f].

    Flat index f = p*8+k maps to A[f//32, f%32]; head r = p//4 and tail
    col = 8*(p%4)+k, so head/tail selection matrices are constants.
    Returns (32, 192) tiles gb = [G1 | G2], hb = [H1 | H2].
    """
    maskb = sb.tile([128, 16], F32, tag="maskb")
    nc.vector.tensor_scalar(out=maskb[:], in0=ab, scalar1=0.49, scalar2=None,
                            op0=ALU.is_ge)
    m3 = maskb[:].rearrange("p (g k) -> p g k", k=8)
    s2 = sb.tile([128, 2], F32, tag="s2")
    nc.vector.tensor_reduce(out=s2[:], in_=m3, axis=AX.X, op=ALU.add)
    pbase_ps = ps.tile([128, 2], F32, tag="psA", bufs=2)
    nc.tensor.matmul(out=pbase_ps[:], lhsT=consts[:, C_SU:C_SU + 128],
                     rhs=s2[:], start=True, stop=True)
    # inclusive prefix along k (8) by doubling, both graphs at once
    a = sb.tile([128, 16], F32, tag="pfa")
    b = sb.tile([128, 16], F32, tag="pfb")
    c = sb.tile([128, 16], F32, tag="pfc")
    a3, b3, c3 = (t[:].rearrange("p (g k) -> p g k", k=8) for t in (a, b, c))
    nc.vector.tensor_copy(out=a3[:, :, 0:1], in_=m3[:, :, 0:1])
    nc.vector.tensor_tensor(out=a3[:, :, 1:8], in0=m3[:, :, 1:8],
                            in1=m3[:, :, 0:7], op=ALU.add)
    nc.vector.tensor_copy(out=b3[:, :, 0:2], in_=a3[:, :, 0:2])
    nc.vector.tensor_tensor(out=b3[:, :, 2:8], in0=a3[:, :, 2:8],
                            in1=a3[:, :, 0:6], op=ALU.add)
    nc.vector.tensor_copy(out=c3[:, :, 0:4], in_=b3[:, :, 0:4])
    nc.vector.tensor_tensor(out=c3[:, :, 4:8], in0=b3[:, :, 4:8],
                            in1=b3[:, :, 0:4], op=ALU.add)
    # rank' = (incl + pbase) * mask - 1
    r0 = sb.tile([128, 16], F32, tag="r0")
    cb_, pb_ = bass.broadcast_tensor_aps(c3, pbase_ps[:, :].unsqueeze(2))
    nc.vector.tensor_tensor(out=r0[:].rearrange("p (g k) -> p g k", k=8),
                            in0=cb_, in1=pb_, op=ALU.add)
    r1 = sb.tile([128, 16], F32, tag="r1")
    nc.vector.tensor_tensor(out=r1[:], in0=r0[:], in1=maskb[:], op=ALU.mult)
    r2 = sb.tile([128, 16], F32, tag="r2")
    nc.vector.tensor_scalar(out=r2[:], in0=r1[:], scalar1=1.0, scalar2=None,
                            op0=ALU.subtract)
    # one-hot per (graph, k): oh[:, 192k + 96g : +96]
    oh = sb.tile([128, 16 * 96], F32, tag="oh")
    for k in range(8):
        for g in range(2):
            nc.vector.tensor_scalar(
                out=oh[:, 192 * k + 96 * g:192 * k + 96 * (g + 1)],
                in0=consts[:, C_IOTA96:C_IOTA96 + 96],
                scalar1=r2[:, 8 * g + k:8 * g + k + 1], scalar2=None,
                op0=ALU.is_equal)
    gps = ps.tile([32, 192], F32, tag="psacc", bufs=4)
    hps = ps.tile([32, 192], F32, tag="psacc", bufs=4)
    for k in range(8):
        nc.tensor.matmul(out=gps[:], lhsT=consts[:, C_SELH:C_SELH + 32],
                         rhs=oh[:, 192 * k:192 * (k + 1)],
                         start=(k == 0), stop=(k == 7))
    for k in range(8):
        nc.tensor.matmul(out=hps[:],
                         lhsT=consts[:, C_SELT + 32 * k:C_SELT + 32 * (k + 1)],
                         rhs=oh[:, 192 * k:192 * (k + 1)],
                         start=(k == 0), stop=(k == 7))
    gb = sb.tile([32, 192], F32, tag="gb")
    hb = sb.tile([32, 192], F32, tag="hb")
    nc.scalar.copy(out=gb[:], in_=gps[:])
    nc.scalar.copy(out=hb[:], in_=hps[:])
    return gb, hb


def build_program(debug: bool = False):
    nc = bacc.Bacc("TRN2", target_bir_lowering=False, debug=debug,
                   num_devices=NCORES)
    big0 = nc.dram_tensor("big0", [128, 336 + C_W], F32, kind="ExternalInput")
    big1 = nc.dram_tensor("big1", [32, 256], F32, kind="ExternalInput")
    cstb = nc.dram_tensor("cstb", [32, CB_W], F16, kind="ExternalInput")
    out = nc.dram_tensor("out", [32768, 32], F32, kind="ExternalOutput")

    with tile.TileContext(nc) as tc:
        with tc.tile_pool(name="sb", bufs=1) as sb, \
             tc.tile_pool(name="ps", bufs=1, space="PSUM") as ps:
            b0 = sb.tile([128, 336 + C_W], F32, tag="b0")
            nc.sync.dma_start(out=b0[:, 0:16], in_=big0[:, 0:16])
            nc.sync.dma_start(out=b0[:, 16:], in_=big0[:, 16:])
            b1 = sb.tile([32, 256], F32, tag="b1")
            nc.sync.dma_start(out=b1[:], in_=big1[:, :])
            ids16 = sb.tile([32, CB_W], F16, tag="ids16")
            nc.sync.dma_start(out=ids16[:], in_=cstb[:, :])
            a1sb, a2sb = b0[:, 0:8], b0[:, 8:16]
            u1sb, u2sb = b0[:, 16:48], b0[:, 48:80]
            l1sb, l2sb = b0[:, 80:208], b0[:, 208:336]
            consts = b0[:, 336:336 + C_W]
            ft1sb, ft2sb = b1[:, 0:128], b1[:, 128:256]
            ident = consts[:, C_ID:C_ID + 128]

            gb, hb = _incidence_both(nc, sb, ps, consts, b0[:, 0:16])
            g1sb, g2sb = gb[:, 0:96], gb[:, 96:192]
            h1sb, h2sb = hb[:, 0:96], hb[:, 96:192]

            # lam_i <- relu(lam_i + lam_i^T)  (symmetric)
            lp = []
            for i, lsb in enumerate((l1sb, l2sb)):
                ltps = ps.tile([128, 128], F32, tag="psA", bufs=2)
                nc.tensor.transpose(out=ltps[:], in_=lsb, identity=ident)
                lpi = sb.tile([128, 128], F32, tag=f"lp{i}")
                nc.vector.tensor_tensor(out=lpi[:], in0=lsb, in1=ltps[:],
                                        op=ALU.add)
                nc.vector.tensor_scalar(out=lpi[:], in0=lpi[:], scalar1=0.0,
                                        scalar2=None, op0=ALU.max)
                lp.append(lpi)

            # edge features X = [F1@G1; F1@H1], Y = [F2@G2; F2@H2] (128, 96)
            feats = {}
            for nm, ft_, gh in (("xg", ft1sb, g1sb), ("xh", ft1sb, h1sb),
                                ("yg", ft2sb, g2sb), ("yh", ft2sb, h2sb)):
                p_ = ps.tile([128, 96], F32, tag="psA", bufs=2)
                nc.tensor.matmul(out=p_[:], lhsT=ft_, rhs=gh,
                                 start=True, stop=True)
                s_ = sb.tile([128, 96], F32, tag=nm)
                nc.scalar.copy(out=s_[:], in_=p_[:])
                feats[nm] = s_

            # T1 = l1p@YG + l2p@YH ; T2 = l2p@YG + l1p@YH
            tts = []
            for i, (la, lb) in enumerate(((lp[0], lp[1]), (lp[1], lp[0]))):
                tp = ps.tile([128, 96], F32, tag="psA", bufs=2)
                nc.tensor.matmul(out=tp[:], lhsT=la[:], rhs=feats["yg"][:],
                                 start=True, stop=False)
                nc.tensor.matmul(out=tp[:], lhsT=lb[:], rhs=feats["yh"][:],
                                 start=False, stop=True)
                ts_ = sb.tile([128, 96], F32, tag=f"t{i}sb")
                nc.scalar.copy(out=ts_[:], in_=tp[:])
                tts.append(ts_)

            # Me[i, j] = sum_d XG[d,i] T1[d,j] + XH[d,i] T2[d,j]  (96, 96)
            meps = ps.tile([96, 96], F32, tag="psA", bufs=2)
            nc.tensor.matmul(out=meps[:], lhsT=feats["xg"][:], rhs=tts[0][:],
                             start=True, stop=False)
            nc.tensor.matmul(out=meps[:], lhsT=feats["xh"][:], rhs=tts[1][:],
                             start=False, stop=True)
            me16 = sb.tile([96, 96], F16, tag="me16")
            nc.scalar.copy(out=me16[:], in_=meps[:])

            # MpT[c, a] = Mp[a, c] = (U2^T U1)[c, a] -> fp16
            mptps = ps.tile([32, 32], F32, tag="psA", bufs=2)
            nc.tensor.matmul(out=mptps[:], lhsT=u2sb, rhs=u1sb,
                             start=True, stop=True)
            mpt16 = sb.tile([32, 32], F16, tag="mpt16")
            nc.scalar.copy(out=mpt16[:], in_=mptps[:])

            # transposes of incidences -> fp16 (96, 32)
            def transp16(src_, tag):
                tps = ps.tile([96, 32], F32, tag="psA", bufs=2)
                nc.tensor.transpose(out=tps[:], in_=src_,
                                    identity=consts[0:32, C_ID:C_ID + 32])
                t16 = sb.tile([96, 32], F16, tag=tag)
                nc.scalar.copy(out=t16[:], in_=tps[:])
                return t16
            g1t16 = transp16(g1sb, "g1t16")
            h1t16 = transp16(h1sb, "h1t16")
            g2t16 = transp16(g2sb, "g2t16")
            h2t16 = transp16(h2sb, "h2t16")

            # P2[e2, (a, b)] = G2T[e2, a] * H2T[e2, b]   (96, 1024) fp16
            p2 = sb.tile([96, 1024], F16, tag="p2")
            g2b, h2b = bass.broadcast_tensor_aps(g2t16[:, :].unsqueeze(2),
                                                 h2t16[:, :].unsqueeze(1))
            nc.vector.tensor_tensor(
                out=p2[:].rearrange("p (a b) -> p a b", b=32),
                in0=g2b, in1=h2b, op=ALU.mult)

            # Z[e1, (a, b)] = sum_e2 Me[e2, e1] P2[e2, (a,b)]  -> fp16
            z16 = sb.tile([96, 1024], F16, tag="z16")
            for h in range(2):
                zps = ps.tile([96, 512], F32, tag="psA", bufs=2)
                nc.tensor.matmul(out=zps[:], lhsT=me16[:],
                                 rhs=p2[:, 512 * h:512 * (h + 1)],
                                 start=True, stop=True)
                nc.scalar.copy(out=z16[:, 512 * h:512 * (h + 1)], in_=zps[:])

            # H1Texp[e1, (c', b)] = H1T[e1, c']   (96, 1024) fp16
            h1exp = sb.tile([96, 1024], F16, tag="h1exp")
            nc.vector.tensor_copy(
                out=h1exp[:].rearrange("p (c b) -> p c b", b=32),
                in_=h1t16[:, :].unsqueeze(2).broadcast_to([96, 32, 32]))

            # D16[c, (a, c')] = eye[c, c'] * MpT[c, a]  (32, 1024) fp16
            d16 = sb.tile([32, 1024], F16, tag="d16")
            eyb, mpb = bass.broadcast_tensor_aps(
                ids16[:, 512:544].unsqueeze(1), mpt16[:, :].unsqueeze(2))
            nc.gpsimd.tensor_tensor(
                out=d16[:].rearrange("p (a c) -> p a c", c=32),
                in0=eyb, in1=mpb, op=ALU.mult)

            # obuf[(q, c), g*1024 + (b, c')] = out row-block alpha = 4g+q
            obuf = sb.tile([128, 8192], F32, tag="obuf")
            for g in range(8):
                # V4 = [V_a for a in 4g..4g+4], each (96, 1024) in (c', b) order
                v4 = sb.tile([96, 4096], F16, tag="v4", bufs=3)
                zap = z16[:, :]
                zin = bass.AP(zap.tensor, zap.offset + 128 * g,
                              [zap.ap[0], [32, 4], [0, 32], [1, 32]])
                hap = h1exp[:, :]
                hin = bass.AP(hap.tensor, hap.offset,
                              [hap.ap[0], [0, 4], [32, 32], [1, 32]])
                veng = nc.vector
                veng.tensor_tensor(
                    out=v4[:].rearrange("p (a c b) -> p a c b", a=4, b=32),
                    in0=zin, in1=hin, op=ALU.mult)
                for h in range(2):
                    pso = ps.tile([128, 512], F32, tag="pso", bufs=2)
                    for q in range(4):
                        alpha = 4 * g + q
                        has_diag = (alpha // 16) == h
                        # rhs: V_alpha read in (b, c') order, b in [16h,16h+16)
                        va = v4[:, 1024 * q:1024 * (q + 1)].rearrange(
                            "p (c b) -> p c b", b=32).transpose([0, 2, 1])
                        nc.tensor.matmul(out=pso[32 * q:32 * (q + 1), :],
                                         lhsT=g1t16[:],
                                         rhs=va[:, 16 * h:16 * (h + 1), :],
                                         start=True, stop=not has_diag,
                                         tile_position=(0, 32 * q))
                        if has_diag:
                            p_ = alpha % 16
                            nc.tensor.matmul(
                                out=pso[32 * q:32 * (q + 1), :],
                                lhsT=d16[:, 32 * alpha:32 * (alpha + 1)],
                                rhs=ids16[:, 512 - 32 * p_:1024 - 32 * p_],
                                start=False, stop=True,
                                tile_position=(0, 32 * q))
                    dst = obuf[:, 1024 * g + 512 * h:1024 * g + 512 * (h + 1)]
                    nc.scalar.copy(out=dst, in_=pso[:])

            # final DMAs: group g covers contiguous out rows [128g, 128g+128)
            # (alpha = 4g+q -> rows (4g+q)*32+c = 128g + 32q + c, and the
            # obuf partition order (q, c) matches the dst row order), so each
            # group writes one fully contiguous 512KB block.
            for g in range(8):
                dst = bass.AP(out, g * 131072, [[1024, 128], [1, 1024]])
                nc.sync.dma_start(out=dst,
                                  in_=obuf[:, 1024 * g:1024 * (g + 1)])
    nc.compile()
    return nc


def make_in_maps(inputs: dict) -> list:
    inputs = {k: np.asarray(v, dtype=np.float32) for k, v in inputs.items()}
    consts, constsb = make_consts()
    in_maps = []
    for b in range(B):
        big0 = np.concatenate([
            inputs["A_src"][b].reshape(128, 8).astype(np.float32),
            inputs["A_tgt"][b].reshape(128, 8).astype(np.float32),
            inputs["U_src"][b].astype(np.float32),
            inputs["U_tgt"][b].astype(np.float32),
            inputs["lambda1"].astype(np.float32),
            inputs["lambda2"].astype(np.float32),
            consts,
        ], axis=1)
        big1 = np.concatenate([
            inputs["F_src"][b].T.astype(np.float32),
            inputs["F_tgt"][b].T.astype(np.float32),
        ], axis=1)
        in_maps.append({
            "big0": np.ascontiguousarray(big0),
            "big1": np.ascontiguousarray(big1),
            "cstb": constsb,
        })
    return in_maps


_NC_CACHE = {}


def kernel(trace: bool = False, **inputs) -> np.ndarray:
    if "nc" not in _NC_CACHE:
        _NC_CACHE["nc"] = build_program()
    nc = _NC_CACHE["nc"]
    in_maps = make_in_maps(inputs)
    res = run_bass_kernel_spmd(nc, in_maps, core_ids=list(range(NCORES)),
                               trace=trace)
    _NC_CACHE["last_results"] = res
    outs = [res.results[b]["out"].reshape(1024, 1024) for b in range(B)]
    return np.stack(outs).astype(np.float32)
